# revision 5
# baseline (speedup 1.0000x reference)
"""Trainium2 Bass kernel for nn_BoundarySeg (segment_reduce).

out[b, j, 0:H]   = sum_{i>=j} A[b, j, i] * h[b, i, :]
out[b, j, H:2H]  = h[b, j, :] * sum_{i>=j} A[b, j, i]

Shapes: A [8, 2048, 2048] f32, h [8, 2048, 256] f32 -> out [8, 2048, 512] f32.
Sharding: data-parallel over batch; core c computes batch c.

Per-core algorithm (L=2048 in 16 tiles of 128, H=256), bf16 pipeline:
  - A's upper-triangular block rows are loaded in 7 large "units" (pairs of
    row-panels, plus one 4-row tail unit) of 1-2 MB each so every DMA runs
    near peak HBM efficiency; units alternate between the two HWDGE rings
    (sync / scalar engines).
  - Each panel's blocks are cast f32->bf16 (split across ACT/DVE/Pool),
    transposed on TensorE in bf16 (fast FWL weight loads, 1 cyc/row), and
    copied PSUM->SBUF in bf16 (DVE 2x mode); the diagonal block is masked
    (keep i >= j) during that copy via a tensor_tensor multiply.
  - h loads once (one 2 MB DMA) as f32, is cast to a bf16 copy with a ones
    column at [.., 256] so the masked row-sum falls out of the main matmul
    as an extra column.
  - acc[j, n] += atT_block^T @ h_ext over i-tiles >= jc (bf16 x bf16 into
    f32 PSUM, N=258). first half = acc[:, 0:256]; second half =
    h_f32[j, :] * acc[:, 256] (ACT activation with per-partition scale).
  - Output is stored as bf16 (2 MB instead of 4 MB; tolerance is 2e-2) in
    one DMA per unit on the SWDGE (gpsimd) queue; the host casts back to
    f32. Matmuls run a few panels behind the transposes so the PE stream
    never blocks on an arriving panel.
"""

import os
import sys

import numpy as np

sys.path.insert(0, "/opt/trn_rl_repo")

import concourse.bass as bass  # noqa: E402
import concourse.bacc as bacc  # noqa: E402
import concourse.tile as tile  # noqa: E402
from concourse import mybir  # noqa: E402
from concourse.bass_utils import run_bass_kernel_spmd  # noqa: E402
from concourse.masks import make_identity, make_lower_triangular  # noqa: E402

B, L, H = 8, 2048, 256
P = 128
NT = L // P
HE = H + 2  # col H = ones (rowsum), col H+1 unused
GROUP = 8  # blocks per cast/transpose/copy group (one PSUM bank in bf16)

F32 = mybir.dt.float32
BF16 = mybir.dt.bfloat16

# Load units: (first row-tile, n row-tiles). Each unit loads the rectangle
# rows [r0, r0+nr) x cols [r0, NT) in one DMA (slight over-fetch below the
# diagonal for rows past the first keeps the transfer large + rectangular).
UNITS = [(0, 2), (2, 2), (4, 2), (6, 2), (8, 2), (10, 2), (12, 4)]

# Results of the last run (exec_time_ns etc.) for the test harness.
LAST_RESULTS = None
_NC_CACHE = {}


def _build_nc():
    nc = bacc.Bacc(None, target_bir_lowering=False)
    a_dram = nc.dram_tensor("a", [L, L], F32, kind="ExternalInput")
    h_dram = nc.dram_tensor("h", [L, H], F32, kind="ExternalInput")
    out_dram = nc.dram_tensor("out", [L, 2 * H], BF16, kind="ExternalOutput")

    with tile.TileContext(nc) as tc:
        with (
            tc.tile_pool(name="const", bufs=1) as const_pool,
            tc.tile_pool(name="hpool", bufs=1) as h_pool,
            tc.tile_pool(name="astage", bufs=3) as a_pool,
            tc.tile_pool(name="abf", bufs=4) as ab_pool,
            tc.tile_pool(name="atT", bufs=5) as at_pool,
            tc.tile_pool(name="tp", bufs=3, space=bass.MemorySpace.PSUM) as tp_pool,
            tc.tile_pool(name="acc", bufs=3, space=bass.MemorySpace.PSUM) as acc_pool,
            tc.tile_pool(name="outsb", bufs=3) as out_pool,
            tc.tile_pool(name="small", bufs=4) as small_pool,
        ):
            identity = const_pool.tile([P, P], BF16)
            make_identity(nc, identity[:])
            # Mask for the *transposed* diagonal block ([i(part), j(free)],
            # keep i >= j -> lower triangular); columns P.. multiply by 1.0.
            cmask = const_pool.tile([P, GROUP * P], BF16)
            make_lower_triangular(nc, cmask[:, 0:P], val=1.0, diag=True)
            nc.vector.memset(cmask[:, P : GROUP * P], 1.0)

            # h: one 2MB DMA (scalar HWDGE ring), then bf16 copy with the
            # ones column; second-half outputs read the f32 copy.
            h_f32 = h_pool.tile([P, NT, H], F32)
            h_ext = h_pool.tile([P, NT, HE], BF16)
            h_re = h_dram[:].rearrange("(t p) n -> p t n", p=P)
            nc.scalar.dma_start(out=h_f32[:], in_=h_re[:])
            nc.vector.memset(h_ext[:, :, H:HE], 1.0)
            half = NT // 2
            nc.vector.tensor_copy(h_ext[:, 0:half, 0:H], h_f32[:, 0:half, :])
            nc.scalar.copy(h_ext[:, half:NT, 0:H], h_f32[:, half:NT, :])

            # Warmup transpose: absorbs the Pool->PE wait for `identity`.
            wtp = tp_pool.tile([P, GROUP * P], BF16, tag="tp")
            nc.tensor.transpose(wtp[:, 0:P], identity[:], identity[:])

            def matmuls_and_store(jc, atT, out_u, t, last_in_unit, store):
                ntiles = NT - jc
                acc = acc_pool.tile([P, HE], F32, tag="acc")
                for k in range(ntiles):
                    nc.tensor.matmul(
                        acc[:],
                        atT[:, k * P : (k + 1) * P],
                        h_ext[:, jc + k, :],
                        start=(k == 0),
                        stop=(k == ntiles - 1),
                    )
                rowsum = small_pool.tile([P, 1], F32, tag="rowsum")
                nc.scalar.copy(rowsum[:], acc[:, H : H + 1])
                nc.vector.tensor_copy(out_u[:, t, 0:H], acc[:, 0:H])
                nc.scalar.activation(
                    out_u[:, t, H : 2 * H],
                    h_f32[:, jc, :],
                    mybir.ActivationFunctionType.Identity,
                    scale=rowsum[:],
                )
                if last_in_unit:
                    store()

            # Cast engines round-robin (ACT carries most; DVE is busy with
            # the PSUM copies; Pool contends only for the port DVE's 2x_1p
            # ops don't use).
            cast_ops = [
                nc.scalar.copy,
                nc.scalar.copy,
                nc.vector.tensor_copy,
                nc.gpsimd.tensor_copy,
                nc.scalar.copy,
                nc.gpsimd.tensor_copy,
            ]
            cast_i = 0
            ring = [nc.sync, nc.scalar]
            pending = []
            for ui, (r0, nr) in enumerate(UNITS):
                w_u = NT - r0  # col tiles loaded for this unit
                a_stage = a_pool.tile([P, nr, w_u * P], F32, tag="astage")
                ring[ui % 2].dma_start(
                    a_stage[:],
                    a_dram[r0 * P : (r0 + nr) * P, r0 * P :].rearrange(
                        "(t p) w -> p t w", p=P
                    ),
                )
                out_u = out_pool.tile([P, nr, 2 * H], BF16, tag="outsb")
                out_slice = out_dram[r0 * P : (r0 + nr) * P, :].rearrange(
                    "(t p) n -> p t n", p=P
                )

                def store(out_u=out_u, out_slice=out_slice):
                    nc.gpsimd.dma_start(out_slice, out_u[:])

                for t in range(nr):
                    jc = r0 + t
                    w_jc = NT - jc
                    skip = jc - r0
                    atT = at_pool.tile([P, w_jc * P], BF16, tag="atT")
                    for g0 in range(0, w_jc, GROUP):
                        gn = min(GROUP, w_jc - g0)
                        ab = ab_pool.tile([P, GROUP * P], BF16, tag="abf")
                        cast_ops[cast_i % len(cast_ops)](
                            ab[:, 0 : gn * P],
                            a_stage[:, t, (skip + g0) * P : (skip + g0 + gn) * P],
                        )
                        cast_i += 1
                        tp = tp_pool.tile([P, GROUP * P], BF16, tag="tp")
                        for k in range(gn):
                            nc.tensor.transpose(
                                tp[:, k * P : (k + 1) * P],
                                ab[:, k * P : (k + 1) * P],
                                identity[:],
                            )
                        if g0 == 0:
                            nc.vector.tensor_tensor(
                                atT[:, 0 : gn * P],
                                tp[:, 0 : gn * P],
                                cmask[:, 0 : gn * P],
                                mybir.AluOpType.mult,
                            )
                        else:
                            nc.vector.tensor_copy(
                                atT[:, g0 * P : (g0 + gn) * P], tp[:, 0 : gn * P]
                            )
                    pending.append((jc, atT, out_u, t, t == nr - 1, store))
                    if len(pending) > 2:
                        matmuls_and_store(*pending.pop(0))

            for item in pending:
                matmuls_and_store(*item)

    nc.finalize()
    return nc


def kernel(span_adjacency, bound_hidden):
    global LAST_RESULTS
    a = np.ascontiguousarray(np.asarray(span_adjacency, dtype=np.float32))
    h = np.ascontiguousarray(np.asarray(bound_hidden, dtype=np.float32))
    assert a.shape == (B, L, L) and h.shape == (B, L, H), (a.shape, h.shape)

    key = "full"
    if key not in _NC_CACHE:
        _NC_CACHE[key] = _build_nc()
    nc = _NC_CACHE[key]

    in_maps = [{"a": a[b], "h": h[b]} for b in range(B)]
    res = run_bass_kernel_spmd(
        nc,
        in_maps,
        core_ids=list(range(B)),
        trace=bool(os.environ.get("KERNEL_TRACE")),
    )
    LAST_RESULTS = res
    out = np.stack(
        [np.asarray(res.results[b]["out"]).astype(np.float32) for b in range(B)],
        axis=0,
    )
    return out
